# revision 4
# baseline (speedup 1.0000x reference)
"""GRU block kernel for Trainium2, 8 NeuronCores, data-parallel over batch.

Problem: x[128,512,1629] f32, W_g[1757,128] (g in r,u,c), b_g[128].
  xproj_g = x @ W_g[128:] + b_g          (big memory-bound GEMM)
  recurrence over T=512:
     r = sigmoid(h @ Wh_r + xr_t); u = sigmoid(h @ Wh_u + xu_t)
     c = tanh((r*h) @ Wh_c + xc_t); h' = (1-u)*u + u*c
Output y[128,512,128] = h_t for all t.

v2 design (per core, B_local=16):
 - All device data in fp16 (PE runs 16-bit at full rate w/ fast weight
   load; fp32 matmuls are ~3.8x slower). PSUM accumulation stays fp32.
 - Host pre-packs x slice as [8 chunks][128 part][13 kb x 1024 m] fp16 so
   each GEMM chunk is ONE contiguous-per-partition 3.3MB DMA (26KB/part).
 - GEMM: 13-step PSUM accumulation per (gate, 512-col half), evicted by
   ScalarE Identity+bias (free bias add) into resident xp[128,T,3,16] fp16.
 - Recurrence: H=128 on partitions, batch on free dim, 2 independent
   8-wide chains interleaved for latency hiding. The xproj additions ride
   on TensorE as identity-matmul PSUM accumulations (start/stop
   has_written trick) instead of costly PSUM-operand DVE adds. Elementwise
   tail (t1/sub/mul/add) is split between VectorE and GpSimd to balance
   engine busy. h state lives directly in the y output ring (fp16),
   streamed to DRAM every 64 steps; host upcasts to fp32.
"""

import numpy as np
from contextlib import ExitStack

import concourse.bass as bass
import concourse.bacc as bacc
import concourse.tile as tile
from concourse import mybir
from concourse import bass_utils

F32 = mybir.dt.float32
F16 = mybir.dt.float16
AF = mybir.ActivationFunctionType

B, T, K, H = 128, 512, 1629, 128
NC = 8
BL = B // NC          # 16 batch per core
M = T * BL            # 8192 flattened (t, b) per core
NKB = 13              # k-blocks of 128 (1664 padded)
KP = NKB * 128
MCD = 1024            # m-cols per GEMM DMA chunk (64 timesteps)
NCH = M // MCD        # 8 chunks
SEG = 64              # recurrence steps per y ring segment
CH = 2                # independent batch chains
BW = BL // CH         # 8

# variant flags (A/B in sim)
RU_ADD = "pe"         # "pe": identity-matmul accumulate; "dve": vector add
C_ADD = "pe"
TAIL_POOL = ("t1", "sub")   # which of t1/sub/mul/add go to GpSimd


def build_program(t_steps=T):
    nc = bacc.Bacc("TRN2", target_bir_lowering=False, debug=False,
                   num_devices=NC)
    xt = nc.dram_tensor("xt", [NCH, 128, NKB * MCD], F16,
                        kind="ExternalInput").ap()
    wx = nc.dram_tensor("wx", [3, NKB, 128, H], F16, kind="ExternalInput").ap()
    wh = nc.dram_tensor("wh", [4, H, H], F16, kind="ExternalInput").ap()
    bz = nc.dram_tensor("bz", [3, H, 1], F32, kind="ExternalInput").ap()
    y = nc.dram_tensor("y", [H, T, BL], F16, kind="ExternalOutput").ap()

    with tile.TileContext(nc) as tc, ExitStack() as ctx:
        consts = ctx.enter_context(tc.tile_pool(name="consts", bufs=1))
        xpp = ctx.enter_context(tc.tile_pool(name="xproj", bufs=1))
        xpool = ctx.enter_context(tc.tile_pool(name="xtiles", bufs=2))
        gpsum = ctx.enter_context(tc.tile_pool(name="gpsum", bufs=2,
                                               space="PSUM"))
        rupsum = ctx.enter_context(tc.tile_pool(name="rupsum", bufs=3,
                                                space="PSUM"))
        cpsum = ctx.enter_context(tc.tile_pool(name="cpsum", bufs=3,
                                               space="PSUM"))
        ypool = ctx.enter_context(tc.tile_pool(name="yring", bufs=2))
        small = ctx.enter_context(tc.tile_pool(name="small", bufs=6))
        state = ctx.enter_context(tc.tile_pool(name="state", bufs=1))

        # ---- load constants ----
        wxt = [[consts.tile([128, H], F16, name=f"wx{g}_{kb}",
                            tag=f"wx{g}_{kb}")
                for kb in range(NKB)] for g in range(3)]
        wht = [consts.tile([H, H], F16, name=f"wh{g}", tag=f"wh{g}")
               for g in range(4)]
        bzt = [consts.tile([H, 1], F32, name=f"bz{g}", tag=f"bz{g}")
               for g in range(3)]
        for g in range(3):
            for kb in range(NKB):
                nc.sync.dma_start(out=wxt[g][kb], in_=wx[g, kb])
            nc.sync.dma_start(out=bzt[g], in_=bz[g])
        for g in range(4):
            nc.sync.dma_start(out=wht[g], in_=wh[g])
        ident = wht[3]

        # resident xproj buffer [128, T, 3, BL] fp16
        xp = xpp.tile([128, T, 3, BL], F16, name="xp", tag="xp")

        # ---- GEMM phase ----
        nseg = (t_steps + SEG - 1) // SEG
        for mc in range(min(NCH, nseg)):
            xtile = xpool.tile([128, NKB, MCD], F16, name="xtile", tag="xtile")
            nc.sync.dma_start(out=xtile, in_=xt[mc])
            t0 = mc * SEG
            for half in range(2):
                for g in range(3):
                    ps = gpsum.tile([128, 32, BL], F32, name="gps", tag="gps")
                    for kb in range(NKB):
                        nc.tensor.matmul(
                            ps, lhsT=wxt[g][kb],
                            rhs=xtile[:, kb, half * 512:(half + 1) * 512],
                            start=(kb == 0), stop=(kb == NKB - 1))
                    th = t0 + half * 32
                    nc.scalar.add(xp[:, th:th + 32, g, :], ps, add=bzt[g])

        # ---- recurrence ----
        h0 = state.tile([128, BL], F16, name="h0", tag="h0")
        nc.vector.memset(h0, 0.0)
        h_prev = [h0[:, w * BW:(w + 1) * BW] for w in range(CH)]

        def vec_or_pool(name):
            return nc.gpsimd if name in TAIL_POOL else nc.vector

        for seg in range(nseg):
            steps = min(SEG, t_steps - seg * SEG)
            yseg = ypool.tile([128, SEG, BL], F16, name="yseg", tag="yseg")
            for tt in range(steps):
                t = seg * SEG + tt
                p_ru = [None] * CH
                # identity-MM seeds PSUM with xproj (depends only on xp, so
                # the scheduler can run it ahead); gate MMs accumulate onto it
                for w in range(CH):
                    p_ru[w] = rupsum.tile([128, 2 * BW], F32, name="pru",
                                          tag="pru")
                    b0 = w * BW
                    xp_ru = xp[:, t, 0:2, b0:b0 + BW]
                    if RU_ADD == "pe":
                        nc.tensor.matmul(p_ru[w], lhsT=ident, rhs=xp_ru,
                                         start=True, stop=False,
                                         skip_group_check=True)
                for w in range(CH):
                    st = RU_ADD != "pe"
                    nc.tensor.matmul(p_ru[w][:, 0:BW], lhsT=wht[0],
                                     rhs=h_prev[w], start=st, stop=True,
                                     skip_group_check=True)
                ru = [None] * CH
                for w in range(CH):
                    st = RU_ADD != "pe"
                    nc.tensor.matmul(p_ru[w][:, BW:2 * BW], lhsT=wht[1],
                                     rhs=h_prev[w], start=st, stop=True,
                                     skip_group_check=True)
                    if RU_ADD != "pe":
                        b0 = w * BW
                        pv = p_ru[w].rearrange("p (g b) -> p g b", g=2)
                        nc.vector.tensor_add(pv, pv, xp[:, t, 0:2, b0:b0 + BW])
                    ru[w] = small.tile([128, 2 * BW], F16, name="ru", tag="ru")
                    nc.scalar.activation(ru[w], p_ru[w], AF.Sigmoid)
                t1 = [None] * CH
                for w in range(CH):
                    t1[w] = small.tile([128, BW], F16, name="t1", tag="t1")
                    vec_or_pool("t1").tensor_mul(t1[w], ru[w][:, 0:BW],
                                                 h_prev[w])
                p_c = [None] * CH
                for w in range(CH):
                    p_c[w] = cpsum.tile([128, BW], F32, name="pc", tag="pc")
                    b0 = w * BW
                    if C_ADD == "pe":
                        nc.tensor.matmul(p_c[w], lhsT=ident,
                                         rhs=xp[:, t, 2, b0:b0 + BW],
                                         start=True, stop=False,
                                         skip_group_check=True)
                for w in range(CH):
                    nc.tensor.matmul(p_c[w], lhsT=wht[2], rhs=t1[w],
                                     start=(C_ADD != "pe"), stop=True,
                                     skip_group_check=True)
                    if C_ADD != "pe":
                        b0 = w * BW
                        nc.vector.tensor_add(p_c[w], p_c[w],
                                             xp[:, t, 2, b0:b0 + BW])
                for w in range(CH):
                    b0 = w * BW
                    hp = h_prev[w]
                    c_t = small.tile([128, BW], F16, name="ct", tag="ct")
                    nc.scalar.activation(c_t, p_c[w], AF.Tanh)
                    d = small.tile([128, BW], F16, name="d", tag="d")
                    vec_or_pool("sub").tensor_sub(d, c_t, hp)
                    mm = small.tile([128, BW], F16, name="mm", tag="mm")
                    vec_or_pool("mul").tensor_mul(mm, ru[w][:, BW:2 * BW], d)
                    h_new = yseg[:, tt, b0:b0 + BW]
                    vec_or_pool("add").tensor_add(h_new, hp, mm)
                    h_prev[w] = h_new
            nc.sync.dma_start(out=y[:, seg * SEG: seg * SEG + steps, :],
                              in_=yseg[:, 0:steps, :])

    nc.compile()
    return nc


def prep_inputs(x, W_r, b_r, W_u, b_u, W_c, b_c):
    """Host-side shard + layout transform. Returns in_maps list for 8 cores."""
    ws = [W_r, W_u, W_c]
    bs = [b_r, b_u, b_c]
    wx = np.zeros((3, NKB, 128, H), dtype=np.float16)
    wh = np.zeros((4, H, H), dtype=np.float16)
    bz = np.zeros((3, H, 1), dtype=np.float32)
    for g in range(3):
        wxa = np.zeros((KP, H), dtype=np.float16)
        wxa[:K] = ws[g][H:].astype(np.float16)
        wx[g] = wxa.reshape(NKB, 128, H)
        wh[g] = ws[g][:H].astype(np.float16)
        bz[g, :, 0] = bs[g]
    wh[3] = np.eye(H, dtype=np.float16)
    in_maps = []
    for c in range(NC):
        xs = x[c * BL:(c + 1) * BL].astype(np.float16)   # [BL, T, K]
        xtc = np.zeros((KP, M), dtype=np.float16)
        # m = t*BL + b ; xt[k, m] = x[b, t, k]
        xtc[:K] = xs.transpose(2, 1, 0).reshape(K, M)
        # repack to [NCH][128 part][NKB, MCD] chunk-major contiguous
        x4 = xtc.reshape(NKB, 128, NCH, MCD)             # kb, p, ch, m
        x4 = x4.transpose(2, 1, 0, 3)                    # ch, p, kb, m
        in_maps.append({
            "xt": np.ascontiguousarray(x4.reshape(NCH, 128, NKB * MCD)),
            "wx": wx, "wh": wh, "bz": bz,
        })
    return in_maps


_CACHED = {}


def kernel(x, W_r, b_r, W_u, b_u, W_c, b_c):
    if "nc" not in _CACHED:
        _CACHED["nc"] = build_program()
    nc = _CACHED["nc"]
    in_maps = prep_inputs(x, W_r, b_r, W_u, b_u, W_c, b_c)
    res = bass_utils.run_bass_kernel_spmd(
        nc, in_maps, core_ids=list(range(NC)), trace=False)
    _CACHED["last_results"] = res
    out = np.empty((B, T, H), dtype=np.float32)
    for c in range(NC):
        yc = res.results[c]["y"]                    # [H, T, BL] fp16
        out[c * BL:(c + 1) * BL] = (
            yc.astype(np.float32).transpose(2, 1, 0))
    return out


# revision 6
# speedup vs baseline: 1.2128x; 1.2128x over previous
"""GRU block kernel for Trainium2, 8 NeuronCores, data-parallel over batch.

Problem: x[128,512,1629] f32, W_g[1757,128] (g in r,u,c), b_g[128].
  xproj_g = x @ W_g[128:] + b_g          (big memory-bound GEMM)
  recurrence over T=512:
     r = sigmoid(h @ Wh_r + xr_t); u = sigmoid(h @ Wh_u + xu_t)
     c = tanh((r*h) @ Wh_c + xc_t); h' = (1-u)*u + u*c
Output y[128,512,128] = h_t for all t.

v2 design (per core, B_local=16):
 - All device data in fp16 (PE runs 16-bit at full rate w/ fast weight
   load; fp32 matmuls are ~3.8x slower). PSUM accumulation stays fp32.
 - Host pre-packs x slice as [8 chunks][128 part][13 kb x 1024 m] fp16 so
   each GEMM chunk is ONE contiguous-per-partition 3.3MB DMA (26KB/part).
 - GEMM: 13-step PSUM accumulation per (gate, 512-col half), evicted by
   ScalarE Identity+bias (free bias add) into resident xp[128,T,3,16] fp16.
 - Recurrence: H=128 on partitions, batch on free dim, 2 independent
   8-wide chains interleaved for latency hiding. The xproj additions ride
   on TensorE as identity-matmul PSUM accumulations (start/stop
   has_written trick) instead of costly PSUM-operand DVE adds. Elementwise
   tail (t1/sub/mul/add) is split between VectorE and GpSimd to balance
   engine busy. h state lives directly in the y output ring (fp16),
   streamed to DRAM every 64 steps; host upcasts to fp32.
"""

import numpy as np
from contextlib import ExitStack

import concourse.bass as bass
import concourse.bacc as bacc
import concourse.tile as tile
from concourse import mybir
from concourse import bass_utils

F32 = mybir.dt.float32
F16 = mybir.dt.float16
AF = mybir.ActivationFunctionType

B, T, K, H = 128, 512, 1629, 128
NC = 8
BL = B // NC          # 16 batch per core
M = T * BL            # 8192 flattened (t, b) per core
NKB = 13              # k-blocks of 128 (1664 padded)
KP = NKB * 128
MCD = 1024            # m-cols per GEMM DMA chunk (64 timesteps)
NCH = M // MCD        # 8 chunks
SEG = 64              # recurrence steps per y ring segment
CH = 2                # independent batch chains
BW = BL // CH         # 8

# variant flags (A/B in sim)
RU_ADD = "pe"         # "pe": identity-matmul accumulate; "dve": vector add
C_ADD = "pe"
TAIL_POOL = ()        # which of t1/sub/mul/add go to GpSimd
PSUM_BUFS = 3


def build_program(t_steps=T):
    ch, bw = CH, BL // CH
    nc = bacc.Bacc("TRN2", target_bir_lowering=False, debug=False,
                   num_devices=NC)
    xt = nc.dram_tensor("xt", [NCH, 128, NKB * MCD], F16,
                        kind="ExternalInput").ap()
    wx = nc.dram_tensor("wx", [3, NKB, 128, H], F16, kind="ExternalInput").ap()
    wh = nc.dram_tensor("wh", [4, H, H], F16, kind="ExternalInput").ap()
    bz = nc.dram_tensor("bz", [3, H, 1], F32, kind="ExternalInput").ap()
    y = nc.dram_tensor("y", [H, T, BL], F16, kind="ExternalOutput").ap()

    with tile.TileContext(nc) as tc, ExitStack() as ctx:
        consts = ctx.enter_context(tc.tile_pool(name="consts", bufs=1))
        xpp = ctx.enter_context(tc.tile_pool(name="xproj", bufs=1))
        xpool = ctx.enter_context(tc.tile_pool(name="xtiles", bufs=2))
        gpsum = ctx.enter_context(tc.tile_pool(name="gpsum", bufs=2,
                                               space="PSUM"))
        rupsum = ctx.enter_context(tc.tile_pool(name="rupsum", bufs=PSUM_BUFS,
                                                space="PSUM"))
        cpsum = ctx.enter_context(tc.tile_pool(name="cpsum", bufs=PSUM_BUFS,
                                               space="PSUM"))
        ypool = ctx.enter_context(tc.tile_pool(name="yring", bufs=2))
        small = ctx.enter_context(tc.tile_pool(name="small", bufs=6))
        state = ctx.enter_context(tc.tile_pool(name="state", bufs=1))

        # ---- load constants ----
        wxt = [[consts.tile([128, H], F16, name=f"wx{g}_{kb}",
                            tag=f"wx{g}_{kb}")
                for kb in range(NKB)] for g in range(3)]
        wht = [consts.tile([H, H], F16, name=f"wh{g}", tag=f"wh{g}")
               for g in range(4)]
        bzt = [consts.tile([H, 1], F32, name=f"bz{g}", tag=f"bz{g}")
               for g in range(3)]
        for g in range(3):
            for kb in range(NKB):
                nc.sync.dma_start(out=wxt[g][kb], in_=wx[g, kb])
            nc.sync.dma_start(out=bzt[g], in_=bz[g])
        for g in range(4):
            nc.sync.dma_start(out=wht[g], in_=wh[g])
        ident = wht[3]

        # resident xproj buffer [128, T, 3, BL] fp16
        xp = xpp.tile([128, T, 3, BL], F16, name="xp", tag="xp")

        # ---- GEMM phase ----
        nseg = (t_steps + SEG - 1) // SEG
        for mc in range(min(NCH, nseg)):
            xtile = xpool.tile([128, NKB, MCD], F16, name="xtile", tag="xtile")
            nc.sync.dma_start(out=xtile, in_=xt[mc])
            t0 = mc * SEG
            for half in range(2):
                for g in range(3):
                    ps = gpsum.tile([128, 32, BL], F32, name="gps", tag="gps")
                    for kb in range(NKB):
                        nc.tensor.matmul(
                            ps, lhsT=wxt[g][kb],
                            rhs=xtile[:, kb, half * 512:(half + 1) * 512],
                            start=(kb == 0), stop=(kb == NKB - 1))
                    th = t0 + half * 32
                    nc.scalar.add(xp[:, th:th + 32, g, :], ps, add=bzt[g])

        # ---- recurrence ----
        h0 = state.tile([128, BL], F16, name="h0", tag="h0")
        nc.vector.memset(h0, 0.0)
        h_prev = [h0[:, w * bw:(w + 1) * bw] for w in range(ch)]

        def vec_or_pool(name):
            return nc.gpsimd if name in TAIL_POOL else nc.vector

        for seg in range(nseg):
            steps = min(SEG, t_steps - seg * SEG)
            yseg = ypool.tile([128, SEG, BL], F16, name="yseg", tag="yseg")
            for tt in range(steps):
                t = seg * SEG + tt
                p_ru = [None] * CH
                # identity-MM seeds PSUM with xproj (depends only on xp, so
                # the scheduler can run it ahead); gate MMs accumulate onto it
                for w in range(ch):
                    p_ru[w] = rupsum.tile([128, 2 * bw], F32, name="pru",
                                          tag="pru")
                    b0 = w * bw
                    xp_ru = xp[:, t, 0:2, b0:b0 + bw]
                    if RU_ADD == "pe":
                        nc.tensor.matmul(p_ru[w], lhsT=ident, rhs=xp_ru,
                                         start=True, stop=False,
                                         skip_group_check=True)
                for w in range(ch):
                    st = RU_ADD != "pe"
                    nc.tensor.matmul(p_ru[w][:, 0:bw], lhsT=wht[0],
                                     rhs=h_prev[w], start=st, stop=True,
                                     skip_group_check=True)
                ru = [None] * CH
                for w in range(ch):
                    st = RU_ADD != "pe"
                    nc.tensor.matmul(p_ru[w][:, bw:2 * bw], lhsT=wht[1],
                                     rhs=h_prev[w], start=st, stop=True,
                                     skip_group_check=True)
                    if RU_ADD != "pe":
                        b0 = w * bw
                        pv = p_ru[w].rearrange("p (g b) -> p g b", g=2)
                        nc.vector.tensor_add(pv, pv, xp[:, t, 0:2, b0:b0 + bw])
                    ru[w] = small.tile([128, 2 * bw], F16, name="ru", tag="ru")
                    nc.scalar.activation(ru[w], p_ru[w], AF.Sigmoid)
                t1 = [None] * CH
                for w in range(ch):
                    t1[w] = small.tile([128, bw], F16, name="t1", tag="t1")
                    vec_or_pool("t1").tensor_mul(t1[w], ru[w][:, 0:bw],
                                                 h_prev[w])
                p_c = [None] * CH
                for w in range(ch):
                    p_c[w] = cpsum.tile([128, bw], F32, name="pc", tag="pc")
                    b0 = w * bw
                    if C_ADD == "pe":
                        nc.tensor.matmul(p_c[w], lhsT=ident,
                                         rhs=xp[:, t, 2, b0:b0 + bw],
                                         start=True, stop=False,
                                         skip_group_check=True)
                for w in range(ch):
                    nc.tensor.matmul(p_c[w], lhsT=wht[2], rhs=t1[w],
                                     start=(C_ADD != "pe"), stop=True,
                                     skip_group_check=True)
                    if C_ADD != "pe":
                        b0 = w * bw
                        nc.vector.tensor_add(p_c[w], p_c[w],
                                             xp[:, t, 2, b0:b0 + bw])
                for w in range(ch):
                    b0 = w * bw
                    hp = h_prev[w]
                    c_t = small.tile([128, bw], F16, name="ct", tag="ct")
                    nc.scalar.activation(c_t, p_c[w], AF.Tanh)
                    d = small.tile([128, bw], F16, name="d", tag="d")
                    vec_or_pool("sub").tensor_sub(d, c_t, hp)
                    mm = small.tile([128, bw], F16, name="mm", tag="mm")
                    vec_or_pool("mul").tensor_mul(mm, ru[w][:, bw:2 * bw], d)
                    h_new = yseg[:, tt, b0:b0 + bw]
                    vec_or_pool("add").tensor_add(h_new, hp, mm)
                    h_prev[w] = h_new
            nc.sync.dma_start(out=y[:, seg * SEG: seg * SEG + steps, :],
                              in_=yseg[:, 0:steps, :])

    nc.compile()
    return nc


def prep_inputs(x, W_r, b_r, W_u, b_u, W_c, b_c):
    """Host-side shard + layout transform. Returns in_maps list for 8 cores."""
    ws = [W_r, W_u, W_c]
    bs = [b_r, b_u, b_c]
    wx = np.zeros((3, NKB, 128, H), dtype=np.float16)
    wh = np.zeros((4, H, H), dtype=np.float16)
    bz = np.zeros((3, H, 1), dtype=np.float32)
    for g in range(3):
        wxa = np.zeros((KP, H), dtype=np.float16)
        wxa[:K] = ws[g][H:].astype(np.float16)
        wx[g] = wxa.reshape(NKB, 128, H)
        wh[g] = ws[g][:H].astype(np.float16)
        bz[g, :, 0] = bs[g]
    wh[3] = np.eye(H, dtype=np.float16)
    in_maps = []
    for c in range(NC):
        xs = x[c * BL:(c + 1) * BL].astype(np.float16)   # [BL, T, K]
        xtc = np.zeros((KP, M), dtype=np.float16)
        # m = t*BL + b ; xt[k, m] = x[b, t, k]
        xtc[:K] = xs.transpose(2, 1, 0).reshape(K, M)
        # repack to [NCH][128 part][NKB, MCD] chunk-major contiguous
        x4 = xtc.reshape(NKB, 128, NCH, MCD)             # kb, p, ch, m
        x4 = x4.transpose(2, 1, 0, 3)                    # ch, p, kb, m
        in_maps.append({
            "xt": np.ascontiguousarray(x4.reshape(NCH, 128, NKB * MCD)),
            "wx": wx, "wh": wh, "bz": bz,
        })
    return in_maps


_CACHED = {}


def kernel(x, W_r, b_r, W_u, b_u, W_c, b_c):
    if "nc" not in _CACHED:
        _CACHED["nc"] = build_program()
    nc = _CACHED["nc"]
    in_maps = prep_inputs(x, W_r, b_r, W_u, b_u, W_c, b_c)
    res = bass_utils.run_bass_kernel_spmd(
        nc, in_maps, core_ids=list(range(NC)), trace=False)
    _CACHED["last_results"] = res
    out = np.empty((B, T, H), dtype=np.float32)
    for c in range(NC):
        yc = res.results[c]["y"]                    # [H, T, BL] fp16
        out[c * BL:(c + 1) * BL] = (
            yc.astype(np.float32).transpose(2, 1, 0))
    return out


# revision 10
# speedup vs baseline: 1.3429x; 1.1072x over previous
"""GRU block kernel for Trainium2, 8 NeuronCores, data-parallel over batch.

Problem: x[128,512,1629] f32, W_g[1757,128] (g in r,u,c), b_g[128].
  xproj_g = x @ W_g[128:] + b_g          (big memory-bound GEMM)
  recurrence over T=512:
     r = sigmoid(h @ Wh_r + xr_t); u = sigmoid(h @ Wh_u + xu_t)
     c = tanh((r*h) @ Wh_c + xc_t); h' = (1-u)*u + u*c
Output y[128,512,128] = h_t for all t.

v2 design (per core, B_local=16):
 - All device data in fp16 (PE runs 16-bit at full rate w/ fast weight
   load; fp32 matmuls are ~3.8x slower). PSUM accumulation stays fp32.
 - Host pre-packs x slice as [8 chunks][128 part][13 kb x 1024 m] fp16 so
   each GEMM chunk is ONE contiguous-per-partition 3.3MB DMA (26KB/part).
 - GEMM: 13-step PSUM accumulation per (gate, 512-col half), evicted by
   ScalarE Identity+bias (free bias add) into resident xp[128,T,3,16] fp16.
 - Recurrence: H=128 on partitions, batch on free dim, 2 independent
   8-wide chains interleaved for latency hiding. The xproj additions ride
   on TensorE as identity-matmul PSUM accumulations (start/stop
   has_written trick) instead of costly PSUM-operand DVE adds. Elementwise
   tail (t1/sub/mul/add) is split between VectorE and GpSimd to balance
   engine busy. h state lives directly in the y output ring (fp16),
   streamed to DRAM every 64 steps; host upcasts to fp32.
"""

import numpy as np
from contextlib import ExitStack

import concourse.bass as bass
import concourse.bacc as bacc
import concourse.tile as tile
from concourse import mybir
from concourse import bass_utils

F32 = mybir.dt.float32
F16 = mybir.dt.float16
AF = mybir.ActivationFunctionType

B, T, K, H = 128, 512, 1629, 128
NC = 8
BL = B // NC          # 16 batch per core
M = T * BL            # 8192 flattened (t, b) per core
NKB = 13              # k-blocks of 128 (1664 padded)
KP = NKB * 128
MCD = 1024            # m-cols per GEMM DMA chunk (64 timesteps)
NCH = M // MCD        # 8 chunks
SEG = 64              # recurrence steps per y ring segment
CH = 2                # independent batch chains
BW = BL // CH         # 8

# variant flags (A/B in sim)
RU_ADD = "pe"         # "pe": identity-matmul accumulate; "dve": vector add
C_ADD = "pe"
TAIL_POOL = ()        # which of t1/sub/mul/add go to GpSimd
GPSUM_BUFS = 2
RU_BUFS = 3
C_BUFS = 3


def build_program(t_steps=T):
    ch, bw = CH, BL // CH
    nc = bacc.Bacc("TRN2", target_bir_lowering=False, debug=False,
                   num_devices=NC)
    xt = nc.dram_tensor("xt", [NCH, 128, NKB * MCD], F16,
                        kind="ExternalInput").ap()
    wx = nc.dram_tensor("wx", [3, NKB, 128, H], F16, kind="ExternalInput").ap()
    wh = nc.dram_tensor("wh", [4, H, H], F16, kind="ExternalInput").ap()
    bz = nc.dram_tensor("bz", [3, H, 1], F32, kind="ExternalInput").ap()
    y = nc.dram_tensor("y", [H, T, BL], F16, kind="ExternalOutput").ap()

    with tile.TileContext(nc) as tc, ExitStack() as ctx:
        consts = ctx.enter_context(tc.tile_pool(name="consts", bufs=1))
        xpp = ctx.enter_context(tc.tile_pool(name="xproj", bufs=1))
        xpool = ctx.enter_context(tc.tile_pool(name="xtiles", bufs=2))
        gpsum = ctx.enter_context(tc.tile_pool(name="gpsum", bufs=GPSUM_BUFS,
                                               space="PSUM"))
        rupsum = ctx.enter_context(tc.tile_pool(name="rupsum", bufs=RU_BUFS,
                                                space="PSUM"))
        cpsum = ctx.enter_context(tc.tile_pool(name="cpsum", bufs=C_BUFS,
                                               space="PSUM"))
        ypool = ctx.enter_context(tc.tile_pool(name="yring", bufs=2))
        small = ctx.enter_context(tc.tile_pool(name="small", bufs=6))
        state = ctx.enter_context(tc.tile_pool(name="state", bufs=1))

        # ---- load constants ----
        wxt = [[consts.tile([128, H], F16, name=f"wx{g}_{kb}",
                            tag=f"wx{g}_{kb}")
                for kb in range(NKB)] for g in range(3)]
        wht = [consts.tile([H, H], F16, name=f"wh{g}", tag=f"wh{g}")
               for g in range(4)]
        bzt = [consts.tile([H, 1], F32, name=f"bz{g}", tag=f"bz{g}")
               for g in range(3)]
        for g in range(3):
            for kb in range(NKB):
                nc.sync.dma_start(out=wxt[g][kb], in_=wx[g, kb])
            nc.sync.dma_start(out=bzt[g], in_=bz[g])
        for g in range(4):
            nc.sync.dma_start(out=wht[g], in_=wh[g])
        ident = wht[3]

        # resident xproj buffer [128, T, 3, BL] fp16
        xp = xpp.tile([128, T, 3, BL], F16, name="xp", tag="xp")

        # ---- GEMM phase ----
        nseg = (t_steps + SEG - 1) // SEG
        tpc = MCD // BL                    # timesteps per GEMM chunk (64)
        for mc in range(min(NCH, (t_steps + tpc - 1) // tpc)):
            xtile = xpool.tile([128, NKB, MCD], F16, name="xtile", tag="xtile")
            nc.sync.dma_start(out=xtile, in_=xt[mc])
            t0 = mc * tpc
            for half in range(2):
                for g in range(3):
                    ps = gpsum.tile([128, 32, BL], F32, name="gps", tag="gps")
                    for kb in range(NKB):
                        nc.tensor.matmul(
                            ps, lhsT=wxt[g][kb],
                            rhs=xtile[:, kb, half * 512:(half + 1) * 512],
                            start=(kb == 0), stop=(kb == NKB - 1))
                    th = t0 + half * 32
                    nc.scalar.add(xp[:, th:th + 32, g, :], ps, add=bzt[g])

        # ---- recurrence ----
        h0 = state.tile([128, BL], F16, name="h0", tag="h0")
        nc.vector.memset(h0, 0.0)
        h_prev = [h0[:, w * bw:(w + 1) * bw] for w in range(ch)]

        def vec_or_pool(name):
            return nc.gpsimd if name in TAIL_POOL else nc.vector

        for seg in range(nseg):
            steps = min(SEG, t_steps - seg * SEG)
            yseg = ypool.tile([128, SEG, BL], F16, name="yseg", tag="yseg")
            for tt in range(steps):
                t = seg * SEG + tt
                p_ru = [None] * CH
                # identity-MM seeds PSUM with xproj (depends only on xp, so
                # the scheduler can run it ahead); gate MMs accumulate onto it
                for w in range(ch):
                    p_ru[w] = rupsum.tile([128, 2 * bw], F32, name="pru",
                                          tag="pru")
                    b0 = w * bw
                    xp_ru = xp[:, t, 0:2, b0:b0 + bw]
                    if RU_ADD == "pe":
                        nc.tensor.matmul(p_ru[w], lhsT=ident, rhs=xp_ru,
                                         start=True, stop=False,
                                         skip_group_check=True)
                for w in range(ch):
                    st = RU_ADD != "pe"
                    nc.tensor.matmul(p_ru[w][:, 0:bw], lhsT=wht[0],
                                     rhs=h_prev[w], start=st, stop=True,
                                     skip_group_check=True)
                ru = [None] * CH
                for w in range(ch):
                    st = RU_ADD != "pe"
                    nc.tensor.matmul(p_ru[w][:, bw:2 * bw], lhsT=wht[1],
                                     rhs=h_prev[w], start=st, stop=True,
                                     skip_group_check=True)
                    if RU_ADD != "pe":
                        b0 = w * bw
                        pv = p_ru[w].rearrange("p (g b) -> p g b", g=2)
                        nc.vector.tensor_add(pv, pv, xp[:, t, 0:2, b0:b0 + bw])
                    ru[w] = small.tile([128, 2 * bw], F16, name="ru", tag="ru")
                    nc.scalar.activation(ru[w], p_ru[w], AF.Sigmoid)
                t1 = [None] * CH
                for w in range(ch):
                    t1[w] = small.tile([128, bw], F16, name="t1", tag="t1")
                    vec_or_pool("t1").tensor_mul(t1[w], ru[w][:, 0:bw],
                                                 h_prev[w])
                p_c = [None] * CH
                for w in range(ch):
                    p_c[w] = cpsum.tile([128, bw], F32, name="pc", tag="pc")
                    b0 = w * bw
                    if C_ADD == "pe":
                        nc.tensor.matmul(p_c[w], lhsT=ident,
                                         rhs=xp[:, t, 2, b0:b0 + bw],
                                         start=True, stop=False,
                                         skip_group_check=True)
                for w in range(ch):
                    nc.tensor.matmul(p_c[w], lhsT=wht[2], rhs=t1[w],
                                     start=(C_ADD != "pe"), stop=True,
                                     skip_group_check=True)
                    if C_ADD != "pe":
                        b0 = w * bw
                        nc.vector.tensor_add(p_c[w], p_c[w],
                                             xp[:, t, 2, b0:b0 + bw])
                for w in range(ch):
                    b0 = w * bw
                    hp = h_prev[w]
                    c_t = small.tile([128, bw], F16, name="ct", tag="ct")
                    nc.scalar.activation(c_t, p_c[w], AF.Tanh)
                    d = small.tile([128, bw], F16, name="d", tag="d")
                    vec_or_pool("sub").tensor_sub(d, c_t, hp)
                    mm = small.tile([128, bw], F16, name="mm", tag="mm")
                    vec_or_pool("mul").tensor_mul(mm, ru[w][:, bw:2 * bw], d)
                    h_new = yseg[:, tt, b0:b0 + bw]
                    vec_or_pool("add").tensor_add(h_new, hp, mm)
                    h_prev[w] = h_new
            nc.sync.dma_start(out=y[:, seg * SEG: seg * SEG + steps, :],
                              in_=yseg[:, 0:steps, :])

    nc.compile()
    return nc


def prep_inputs(x, W_r, b_r, W_u, b_u, W_c, b_c):
    """Host-side shard + layout transform. Returns in_maps list for 8 cores."""
    ws = [W_r, W_u, W_c]
    bs = [b_r, b_u, b_c]
    wx = np.zeros((3, NKB, 128, H), dtype=np.float16)
    wh = np.zeros((4, H, H), dtype=np.float16)
    bz = np.zeros((3, H, 1), dtype=np.float32)
    for g in range(3):
        wxa = np.zeros((KP, H), dtype=np.float16)
        wxa[:K] = ws[g][H:].astype(np.float16)
        wx[g] = wxa.reshape(NKB, 128, H)
        wh[g] = ws[g][:H].astype(np.float16)
        bz[g, :, 0] = bs[g]
    wh[3] = np.eye(H, dtype=np.float16)
    in_maps = []
    for c in range(NC):
        xs = x[c * BL:(c + 1) * BL].astype(np.float16)   # [BL, T, K]
        xtc = np.zeros((KP, M), dtype=np.float16)
        # m = t*BL + b ; xt[k, m] = x[b, t, k]
        xtc[:K] = xs.transpose(2, 1, 0).reshape(K, M)
        # repack to [NCH][128 part][NKB, MCD] chunk-major contiguous
        x4 = xtc.reshape(NKB, 128, NCH, MCD)             # kb, p, ch, m
        x4 = x4.transpose(2, 1, 0, 3)                    # ch, p, kb, m
        in_maps.append({
            "xt": np.ascontiguousarray(x4.reshape(NCH, 128, NKB * MCD)),
            "wx": wx, "wh": wh, "bz": bz,
        })
    return in_maps


_CACHED = {}


def kernel(x, W_r, b_r, W_u, b_u, W_c, b_c):
    if "nc" not in _CACHED:
        _CACHED["nc"] = build_program()
    nc = _CACHED["nc"]
    in_maps = prep_inputs(x, W_r, b_r, W_u, b_u, W_c, b_c)
    res = bass_utils.run_bass_kernel_spmd(
        nc, in_maps, core_ids=list(range(NC)), trace=False)
    _CACHED["last_results"] = res
    out = np.empty((B, T, H), dtype=np.float32)
    for c in range(NC):
        yc = res.results[c]["y"]                    # [H, T, BL] fp16
        out[c * BL:(c + 1) * BL] = (
            yc.astype(np.float32).transpose(2, 1, 0))
    return out


# revision 15
# speedup vs baseline: 1.5151x; 1.1282x over previous
"""GRU block kernel for Trainium2, 8 NeuronCores, data-parallel over batch.

Problem: x[128,512,1629] f32, W_g[1757,128] (g in r,u,c), b_g[128].
  xproj_g = x @ W_g[128:] + b_g          (big memory-bound GEMM)
  recurrence over T=512:
     r = sigmoid(h @ Wh_r + xr_t); u = sigmoid(h @ Wh_u + xu_t)
     c = tanh((r*h) @ Wh_c + xc_t); h' = (1-u)*u + u*c
Output y[128,512,128] = h_t for all t.

v2 design (per core, B_local=16):
 - All device data in fp16 (PE runs 16-bit at full rate w/ fast weight
   load; fp32 matmuls are ~3.8x slower). PSUM accumulation stays fp32.
 - Host pre-packs x slice as [8 chunks][128 part][13 kb x 1024 m] fp16 so
   each GEMM chunk is ONE contiguous-per-partition 3.3MB DMA (26KB/part).
 - GEMM: 13-step PSUM accumulation per (gate, 512-col half), evicted by
   ScalarE Identity+bias (free bias add) into resident xp[128,T,3,16] fp16.
 - Recurrence: H=128 on partitions, batch on free dim, 2 independent
   8-wide chains interleaved for latency hiding. The xproj additions ride
   on TensorE as identity-matmul PSUM accumulations (start/stop
   has_written trick) instead of costly PSUM-operand DVE adds. Elementwise
   tail (t1/sub/mul/add) is split between VectorE and GpSimd to balance
   engine busy. h state lives directly in the y output ring (fp16),
   streamed to DRAM every 64 steps; host upcasts to fp32.
"""

import numpy as np
from contextlib import ExitStack

import concourse.bass as bass
import concourse.bacc as bacc
import concourse.tile as tile
from concourse import mybir
from concourse import bass_utils

F32 = mybir.dt.float32
F16 = mybir.dt.float16
AF = mybir.ActivationFunctionType

B, T, K, H = 128, 512, 1629, 128
NC = 8
BL = B // NC          # 16 batch per core
M = T * BL            # 8192 flattened (t, b) per core
NKB = 13              # k-blocks of 128 (1664 padded)
KP = NKB * 128
MCD = 1024            # m-cols per GEMM DMA chunk (64 timesteps)
NCH = M // MCD        # 8 chunks
SEG = 64              # recurrence steps per y ring segment
CH = 2                # independent batch chains
BW = BL // CH         # 8

# variant flags (A/B in sim)
RU_ADD = "pe"         # "pe": identity-matmul accumulate; "dve": vector add
C_ADD = "pe"
TAIL_POOL = ()        # which of t1/sub/mul/add go to GpSimd
TAIL = "short"        # "short": h' = (h - u*h) + u*c ; "classic": h + u*(c-h)
GPSUM_BUFS = 2
SMALL_BUFS = 6
XT_BUFS = 2
RU_BUFS = 3
C_BUFS = 3


def build_program(t_steps=T):
    ch, bw = CH, BL // CH
    nc = bacc.Bacc("TRN2", target_bir_lowering=False, debug=False,
                   num_devices=NC)
    xt = nc.dram_tensor("xt", [NCH, 128, NKB * MCD], F16,
                        kind="ExternalInput").ap()
    wx = nc.dram_tensor("wx", [3, NKB, 128, H], F16, kind="ExternalInput").ap()
    wh = nc.dram_tensor("wh", [4, H, H], F16, kind="ExternalInput").ap()
    bz = nc.dram_tensor("bz", [3, H, 1], F32, kind="ExternalInput").ap()
    y = nc.dram_tensor("y", [H, T, BL], F16, kind="ExternalOutput").ap()

    with tile.TileContext(nc) as tc, ExitStack() as ctx:
        consts = ctx.enter_context(tc.tile_pool(name="consts", bufs=1))
        xpp = ctx.enter_context(tc.tile_pool(name="xproj", bufs=1))
        xpool = ctx.enter_context(tc.tile_pool(name="xtiles", bufs=XT_BUFS))
        gpsum = ctx.enter_context(tc.tile_pool(name="gpsum", bufs=GPSUM_BUFS,
                                               space="PSUM"))
        rupsum = ctx.enter_context(tc.tile_pool(name="rupsum", bufs=RU_BUFS,
                                                space="PSUM"))
        cpsum = ctx.enter_context(tc.tile_pool(name="cpsum", bufs=C_BUFS,
                                               space="PSUM"))
        ypool = ctx.enter_context(tc.tile_pool(name="yring", bufs=2))
        small = ctx.enter_context(tc.tile_pool(name="small", bufs=SMALL_BUFS))
        state = ctx.enter_context(tc.tile_pool(name="state", bufs=1))

        # ---- load constants ----
        wxt = [[consts.tile([128, H], F16, name=f"wx{g}_{kb}",
                            tag=f"wx{g}_{kb}")
                for kb in range(NKB)] for g in range(3)]
        wht = [consts.tile([H, H], F16, name=f"wh{g}", tag=f"wh{g}")
               for g in range(4)]
        bzt = [consts.tile([H, 1], F32, name=f"bz{g}", tag=f"bz{g}")
               for g in range(3)]
        for g in range(3):
            for kb in range(NKB):
                nc.sync.dma_start(out=wxt[g][kb], in_=wx[g, kb])
            nc.sync.dma_start(out=bzt[g], in_=bz[g])
        for g in range(4):
            nc.sync.dma_start(out=wht[g], in_=wh[g])
        ident = wht[3]

        # resident xproj buffer [128, T, 3, BL] fp16
        xp = xpp.tile([128, T, 3, BL], F16, name="xp", tag="xp")

        # ---- GEMM phase ----
        nseg = (t_steps + SEG - 1) // SEG
        tpc = MCD // BL                    # timesteps per GEMM chunk (64)
        for mc in range(min(NCH, (t_steps + tpc - 1) // tpc)):
            xtile = xpool.tile([128, NKB, MCD], F16, name="xtile", tag="xtile")
            nc.sync.dma_start(out=xtile, in_=xt[mc])
            t0 = mc * tpc
            for half in range(2):
                for g in range(3):
                    ps = gpsum.tile([128, 32, BL], F32, name="gps", tag="gps")
                    for kb in range(NKB):
                        nc.tensor.matmul(
                            ps, lhsT=wxt[g][kb],
                            rhs=xtile[:, kb, half * 512:(half + 1) * 512],
                            start=(kb == 0), stop=(kb == NKB - 1))
                    th = t0 + half * 32
                    nc.scalar.add(xp[:, th:th + 32, g, :], ps, add=bzt[g])

        # ---- recurrence ----
        h0 = state.tile([128, BL], F16, name="h0", tag="h0")
        nc.vector.memset(h0, 0.0)
        h_prev = [h0[:, w * bw:(w + 1) * bw] for w in range(ch)]

        def vec_or_pool(name):
            return nc.gpsimd if name in TAIL_POOL else nc.vector

        for seg in range(nseg):
            steps = min(SEG, t_steps - seg * SEG)
            yseg = ypool.tile([128, SEG, BL], F16, name="yseg", tag="yseg")
            for tt in range(steps):
                t = seg * SEG + tt
                p_ru = [None] * CH
                # identity-MM seeds PSUM with xproj (depends only on xp, so
                # the scheduler can run it ahead); gate MMs accumulate onto it
                for w in range(ch):
                    p_ru[w] = rupsum.tile([128, 2 * bw], F32, name="pru",
                                          tag="pru")
                    b0 = w * bw
                    xp_ru = xp[:, t, 0:2, b0:b0 + bw]
                    if RU_ADD == "pe":
                        nc.tensor.matmul(p_ru[w], lhsT=ident, rhs=xp_ru,
                                         start=True, stop=False,
                                         skip_group_check=True)
                for w in range(ch):
                    st = RU_ADD != "pe"
                    nc.tensor.matmul(p_ru[w][:, 0:bw], lhsT=wht[0],
                                     rhs=h_prev[w], start=st, stop=True,
                                     skip_group_check=True)
                ru = [None] * CH
                for w in range(ch):
                    st = RU_ADD != "pe"
                    nc.tensor.matmul(p_ru[w][:, bw:2 * bw], lhsT=wht[1],
                                     rhs=h_prev[w], start=st, stop=True,
                                     skip_group_check=True)
                    if RU_ADD != "pe":
                        b0 = w * bw
                        pv = p_ru[w].rearrange("p (g b) -> p g b", g=2)
                        nc.vector.tensor_add(pv, pv, xp[:, t, 0:2, b0:b0 + bw])
                    ru[w] = small.tile([128, 2 * bw], F16, name="ru", tag="ru")
                    nc.scalar.activation(ru[w], p_ru[w], AF.Sigmoid)
                t1 = [None] * CH
                for w in range(ch):
                    t1[w] = small.tile([128, bw], F16, name="t1", tag="t1")
                    vec_or_pool("t1").tensor_mul(t1[w], ru[w][:, 0:bw],
                                                 h_prev[w])
                g = [None] * CH
                if TAIL == "short":
                    # g = h - u*h ready right after sigmoid -> off the
                    # critical path; post-tanh tail shrinks to 2 ops
                    for w in range(ch):
                        ud = small.tile([128, bw], F16, name="ud", tag="ud")
                        vec_or_pool("ud").tensor_mul(ud,
                                                     ru[w][:, bw:2 * bw],
                                                     h_prev[w])
                        g[w] = small.tile([128, bw], F16, name="g", tag="g")
                        vec_or_pool("g").scalar_tensor_tensor(
                            g[w], ud, -1.0, h_prev[w],
                            mybir.AluOpType.mult, mybir.AluOpType.add)
                p_c = [None] * CH
                for w in range(ch):
                    p_c[w] = cpsum.tile([128, bw], F32, name="pc", tag="pc")
                    b0 = w * bw
                    if C_ADD == "pe":
                        nc.tensor.matmul(p_c[w], lhsT=ident,
                                         rhs=xp[:, t, 2, b0:b0 + bw],
                                         start=True, stop=False,
                                         skip_group_check=True)
                for w in range(ch):
                    nc.tensor.matmul(p_c[w], lhsT=wht[2], rhs=t1[w],
                                     start=(C_ADD != "pe"), stop=True,
                                     skip_group_check=True)
                    if C_ADD != "pe":
                        b0 = w * bw
                        nc.vector.tensor_add(p_c[w], p_c[w],
                                             xp[:, t, 2, b0:b0 + bw])
                for w in range(ch):
                    b0 = w * bw
                    hp = h_prev[w]
                    c_t = small.tile([128, bw], F16, name="ct", tag="ct")
                    nc.scalar.activation(c_t, p_c[w], AF.Tanh)
                    h_new = yseg[:, tt, b0:b0 + bw]
                    if TAIL == "short":
                        # h' = g + u*c  (2 ops after tanh)
                        m1 = small.tile([128, bw], F16, name="mm", tag="mm")
                        nc.vector.tensor_mul(m1, ru[w][:, bw:2 * bw], c_t)
                        nc.vector.tensor_add(h_new, g[w], m1)
                    else:
                        d = small.tile([128, bw], F16, name="d", tag="d")
                        vec_or_pool("sub").tensor_sub(d, c_t, hp)
                        mm = small.tile([128, bw], F16, name="mm", tag="mm")
                        vec_or_pool("mul").tensor_mul(mm, ru[w][:, bw:2 * bw],
                                                      d)
                        vec_or_pool("add").tensor_add(h_new, hp, mm)
                    h_prev[w] = h_new
            nc.sync.dma_start(out=y[:, seg * SEG: seg * SEG + steps, :],
                              in_=yseg[:, 0:steps, :])

    nc.compile()
    return nc


def prep_inputs(x, W_r, b_r, W_u, b_u, W_c, b_c):
    """Host-side shard + layout transform. Returns in_maps list for 8 cores."""
    ws = [W_r, W_u, W_c]
    bs = [b_r, b_u, b_c]
    wx = np.zeros((3, NKB, 128, H), dtype=np.float16)
    wh = np.zeros((4, H, H), dtype=np.float16)
    bz = np.zeros((3, H, 1), dtype=np.float32)
    for g in range(3):
        wxa = np.zeros((KP, H), dtype=np.float16)
        wxa[:K] = ws[g][H:].astype(np.float16)
        wx[g] = wxa.reshape(NKB, 128, H)
        wh[g] = ws[g][:H].astype(np.float16)
        bz[g, :, 0] = bs[g]
    wh[3] = np.eye(H, dtype=np.float16)
    in_maps = []
    for c in range(NC):
        xs = x[c * BL:(c + 1) * BL].astype(np.float16)   # [BL, T, K]
        xtc = np.zeros((KP, M), dtype=np.float16)
        # m = t*BL + b ; xt[k, m] = x[b, t, k]
        xtc[:K] = xs.transpose(2, 1, 0).reshape(K, M)
        # repack to [NCH][128 part][NKB, MCD] chunk-major contiguous
        x4 = xtc.reshape(NKB, 128, NCH, MCD)             # kb, p, ch, m
        x4 = x4.transpose(2, 1, 0, 3)                    # ch, p, kb, m
        in_maps.append({
            "xt": np.ascontiguousarray(x4.reshape(NCH, 128, NKB * MCD)),
            "wx": wx, "wh": wh, "bz": bz,
        })
    return in_maps


_CACHED = {}


def kernel(x, W_r, b_r, W_u, b_u, W_c, b_c):
    if "nc" not in _CACHED:
        _CACHED["nc"] = build_program()
    nc = _CACHED["nc"]
    in_maps = prep_inputs(x, W_r, b_r, W_u, b_u, W_c, b_c)
    res = bass_utils.run_bass_kernel_spmd(
        nc, in_maps, core_ids=list(range(NC)), trace=False)
    _CACHED["last_results"] = res
    out = np.empty((B, T, H), dtype=np.float32)
    for c in range(NC):
        yc = res.results[c]["y"]                    # [H, T, BL] fp16
        out[c * BL:(c + 1) * BL] = (
            yc.astype(np.float32).transpose(2, 1, 0))
    return out


# revision 17
# speedup vs baseline: 1.6561x; 1.0930x over previous
"""GRU block kernel for Trainium2, 8 NeuronCores, data-parallel over batch.

Problem: x[128,512,1629] f32, W_g[1757,128] (g in r,u,c), b_g[128].
  xproj_g = x @ W_g[128:] + b_g          (big memory-bound GEMM)
  recurrence over T=512:
     r = sigmoid(h @ Wh_r + xr_t); u = sigmoid(h @ Wh_u + xu_t)
     c = tanh((r*h) @ Wh_c + xc_t); h' = (1-u)*u + u*c
Output y[128,512,128] = h_t for all t.

v2 design (per core, B_local=16):
 - All device data in fp16 (PE runs 16-bit at full rate w/ fast weight
   load; fp32 matmuls are ~3.8x slower). PSUM accumulation stays fp32.
 - Host pre-packs x slice as [8 chunks][128 part][13 kb x 1024 m] fp16 so
   each GEMM chunk is ONE contiguous-per-partition 3.3MB DMA (26KB/part).
 - GEMM: 13-step PSUM accumulation per (gate, 512-col half), evicted by
   ScalarE Identity+bias (free bias add) into resident xp[128,T,3,16] fp16.
 - Recurrence: H=128 on partitions, batch on free dim, 2 independent
   8-wide chains interleaved for latency hiding. The xproj additions ride
   on TensorE as identity-matmul PSUM seeds (identity-MM start=True writes
   xproj first and depends only on data ready a segment ahead; gate MMs
   accumulate onto it) instead of costly PSUM-operand DVE adds. The
   h-update uses the path-shortened form h' = (h - u*h) + u*c: g = h - u*h
   is computed right after sigmoid (mul + fused scalar_tensor_tensor, off
   the critical path), leaving only 2 DVE ops after tanh. h state lives
   directly in the y output ring (fp16), streamed to DRAM every 64 steps;
   host upcasts to fp32.
"""

import numpy as np
from contextlib import ExitStack

import concourse.bass as bass
import concourse.bacc as bacc
import concourse.tile as tile
from concourse import mybir
from concourse import bass_utils

F32 = mybir.dt.float32
F16 = mybir.dt.float16
AF = mybir.ActivationFunctionType

B, T, K, H = 128, 512, 1629, 128
NC = 8
BL = B // NC          # 16 batch per core
M = T * BL            # 8192 flattened (t, b) per core
NKB = 13              # k-blocks of 128 (1664 padded)
KP = NKB * 128
MCD = 1024            # m-cols per GEMM DMA chunk (64 timesteps)
NCH = M // MCD        # 8 chunks
SEG = 64              # recurrence steps per y ring segment
CH = 2                # independent batch chains
BW = BL // CH         # 8

# variant flags (A/B in sim)
RU_ADD = "pe"         # "pe": identity-matmul accumulate; "dve": vector add
C_ADD = "pe"
TAIL_POOL = ()        # which of t1/sub/mul/add go to GpSimd
TAIL = "short"        # "short": h' = (h - u*h) + u*c ; "classic": h + u*(c-h)
GPSUM_BUFS = 2
SMALL_BUFS = 6
XT_BUFS = 2
EMIT = "step"
PSUM_PER_CHAIN = False
RU_BUFS = 3
C_BUFS = 3


def build_program(t_steps=T):
    ch, bw = CH, BL // CH
    nc = bacc.Bacc("TRN2", target_bir_lowering=False, debug=False,
                   num_devices=NC)
    xt = nc.dram_tensor("xt", [NCH, 128, NKB * MCD], F16,
                        kind="ExternalInput").ap()
    wx = nc.dram_tensor("wx", [3, NKB, 128, H], F16, kind="ExternalInput").ap()
    wh = nc.dram_tensor("wh", [4, H, H], F16, kind="ExternalInput").ap()
    bz = nc.dram_tensor("bz", [3, H, 1], F32, kind="ExternalInput").ap()
    y = nc.dram_tensor("y", [H, T, BL], F16, kind="ExternalOutput").ap()

    with tile.TileContext(nc) as tc, ExitStack() as ctx:
        consts = ctx.enter_context(tc.tile_pool(name="consts", bufs=1))
        xpp = ctx.enter_context(tc.tile_pool(name="xproj", bufs=1))
        xpool = ctx.enter_context(tc.tile_pool(name="xtiles", bufs=XT_BUFS))
        gpsum = ctx.enter_context(tc.tile_pool(name="gpsum", bufs=GPSUM_BUFS,
                                               space="PSUM"))
        rupsum = ctx.enter_context(tc.tile_pool(name="rupsum", bufs=RU_BUFS,
                                                space="PSUM"))
        cpsum = ctx.enter_context(tc.tile_pool(name="cpsum", bufs=C_BUFS,
                                               space="PSUM"))
        ypool = ctx.enter_context(tc.tile_pool(name="yring", bufs=2))
        small = ctx.enter_context(tc.tile_pool(name="small", bufs=SMALL_BUFS))
        state = ctx.enter_context(tc.tile_pool(name="state", bufs=1))

        # ---- load constants ----
        wxt = [[consts.tile([128, H], F16, name=f"wx{g}_{kb}",
                            tag=f"wx{g}_{kb}")
                for kb in range(NKB)] for g in range(3)]
        wht = [consts.tile([H, H], F16, name=f"wh{g}", tag=f"wh{g}")
               for g in range(4)]
        bzt = [consts.tile([H, 1], F32, name=f"bz{g}", tag=f"bz{g}")
               for g in range(3)]
        for g in range(3):
            for kb in range(NKB):
                nc.sync.dma_start(out=wxt[g][kb], in_=wx[g, kb])
            nc.sync.dma_start(out=bzt[g], in_=bz[g])
        for g in range(4):
            nc.sync.dma_start(out=wht[g], in_=wh[g])
        ident = wht[3]

        # resident xproj buffer [128, T, 3, BL] fp16
        xp = xpp.tile([128, T, 3, BL], F16, name="xp", tag="xp")

        # ---- GEMM phase ----
        nseg = (t_steps + SEG - 1) // SEG
        tpc = MCD // BL                    # timesteps per GEMM chunk (64)
        for mc in range(min(NCH, (t_steps + tpc - 1) // tpc)):
            xtile = xpool.tile([128, NKB, MCD], F16, name="xtile", tag="xtile")
            nc.sync.dma_start(out=xtile, in_=xt[mc])
            t0 = mc * tpc
            for half in range(2):
                for g in range(3):
                    ps = gpsum.tile([128, 32, BL], F32, name="gps", tag="gps")
                    for kb in range(NKB):
                        nc.tensor.matmul(
                            ps, lhsT=wxt[g][kb],
                            rhs=xtile[:, kb, half * 512:(half + 1) * 512],
                            start=(kb == 0), stop=(kb == NKB - 1))
                    th = t0 + half * 32
                    nc.scalar.add(xp[:, th:th + 32, g, :], ps, add=bzt[g])

        # ---- recurrence ----
        h0 = state.tile([128, BL], F16, name="h0", tag="h0")
        nc.vector.memset(h0, 0.0)
        h_prev = [h0[:, w * bw:(w + 1) * bw] for w in range(ch)]

        def vec_or_pool(name):
            return nc.gpsimd if name in TAIL_POOL else nc.vector

        def emit_step(w, t, tt, yseg):
            b0 = w * bw
            ptag = f"pru{w}" if PSUM_PER_CHAIN else "pru"
            ctag = f"pc{w}" if PSUM_PER_CHAIN else "pc"
            p_ru = rupsum.tile([128, 2 * bw], F32, name="pru", tag=ptag)
            if RU_ADD == "pe":
                nc.tensor.matmul(p_ru, lhsT=ident,
                                 rhs=xp[:, t, 0:2, b0:b0 + bw],
                                 start=True, stop=False,
                                 skip_group_check=True)
            st = RU_ADD != "pe"
            nc.tensor.matmul(p_ru[:, 0:bw], lhsT=wht[0], rhs=h_prev[w],
                             start=st, stop=True, skip_group_check=True)
            nc.tensor.matmul(p_ru[:, bw:2 * bw], lhsT=wht[1], rhs=h_prev[w],
                             start=st, stop=True, skip_group_check=True)
            if RU_ADD != "pe":
                pv = p_ru.rearrange("p (g b) -> p g b", g=2)
                nc.vector.tensor_add(pv, pv, xp[:, t, 0:2, b0:b0 + bw])
            ru = small.tile([128, 2 * bw], F16, name="ru", tag=f"ru{w}")
            nc.scalar.activation(ru, p_ru, AF.Sigmoid)
            t1 = small.tile([128, bw], F16, name="t1", tag=f"t1{w}")
            vec_or_pool("t1").tensor_mul(t1, ru[:, 0:bw], h_prev[w])
            g = None
            if TAIL == "short":
                # g = h - u*h ready right after sigmoid -> off the critical
                # path; post-tanh tail shrinks to 2 ops
                ud = small.tile([128, bw], F16, name="ud", tag=f"ud{w}")
                vec_or_pool("ud").tensor_mul(ud, ru[:, bw:2 * bw], h_prev[w])
                g = small.tile([128, bw], F16, name="g", tag=f"g{w}")
                vec_or_pool("g").scalar_tensor_tensor(
                    g, ud, -1.0, h_prev[w],
                    mybir.AluOpType.mult, mybir.AluOpType.add)
            p_c = cpsum.tile([128, bw], F32, name="pc", tag=ctag)
            if C_ADD == "pe":
                nc.tensor.matmul(p_c, lhsT=ident,
                                 rhs=xp[:, t, 2, b0:b0 + bw],
                                 start=True, stop=False,
                                 skip_group_check=True)
            nc.tensor.matmul(p_c, lhsT=wht[2], rhs=t1,
                             start=(C_ADD != "pe"), stop=True,
                             skip_group_check=True)
            if C_ADD != "pe":
                nc.vector.tensor_add(p_c, p_c, xp[:, t, 2, b0:b0 + bw])
            c_t = small.tile([128, bw], F16, name="ct", tag=f"ct{w}")
            nc.scalar.activation(c_t, p_c, AF.Tanh)
            h_new = yseg[:, tt, b0:b0 + bw]
            if TAIL == "short":
                m1 = small.tile([128, bw], F16, name="mm", tag=f"mm{w}")
                nc.vector.tensor_mul(m1, ru[:, bw:2 * bw], c_t)
                nc.vector.tensor_add(h_new, g, m1)
            else:
                d = small.tile([128, bw], F16, name="d", tag=f"d{w}")
                vec_or_pool("sub").tensor_sub(d, c_t, h_prev[w])
                mm = small.tile([128, bw], F16, name="mm", tag=f"mm{w}")
                vec_or_pool("mul").tensor_mul(mm, ru[:, bw:2 * bw], d)
                vec_or_pool("add").tensor_add(h_new, h_prev[w], mm)
            h_prev[w] = h_new

        for seg in range(nseg):
            steps = min(SEG, t_steps - seg * SEG)
            yseg = ypool.tile([128, SEG, BL], F16, name="yseg", tag="yseg")
            if EMIT == "chain":
                for w in range(ch):
                    for tt in range(steps):
                        emit_step(w, seg * SEG + tt, tt, yseg)
            else:
                for tt in range(steps):
                    for w in range(ch):
                        emit_step(w, seg * SEG + tt, tt, yseg)
            nc.sync.dma_start(out=y[:, seg * SEG: seg * SEG + steps, :],
                              in_=yseg[:, 0:steps, :])

    nc.compile()
    return nc


def prep_inputs(x, W_r, b_r, W_u, b_u, W_c, b_c):
    """Host-side shard + layout transform. Returns in_maps list for 8 cores."""
    ws = [W_r, W_u, W_c]
    bs = [b_r, b_u, b_c]
    wx = np.zeros((3, NKB, 128, H), dtype=np.float16)
    wh = np.zeros((4, H, H), dtype=np.float16)
    bz = np.zeros((3, H, 1), dtype=np.float32)
    for g in range(3):
        wxa = np.zeros((KP, H), dtype=np.float16)
        wxa[:K] = ws[g][H:].astype(np.float16)
        wx[g] = wxa.reshape(NKB, 128, H)
        wh[g] = ws[g][:H].astype(np.float16)
        bz[g, :, 0] = bs[g]
    wh[3] = np.eye(H, dtype=np.float16)
    in_maps = []
    for c in range(NC):
        xs = x[c * BL:(c + 1) * BL].astype(np.float16)   # [BL, T, K]
        xtc = np.zeros((KP, M), dtype=np.float16)
        # m = t*BL + b ; xt[k, m] = x[b, t, k]
        xtc[:K] = xs.transpose(2, 1, 0).reshape(K, M)
        # repack to [NCH][128 part][NKB, MCD] chunk-major contiguous
        x4 = xtc.reshape(NKB, 128, NCH, MCD)             # kb, p, ch, m
        x4 = x4.transpose(2, 1, 0, 3)                    # ch, p, kb, m
        in_maps.append({
            "xt": np.ascontiguousarray(x4.reshape(NCH, 128, NKB * MCD)),
            "wx": wx, "wh": wh, "bz": bz,
        })
    return in_maps


_CACHED = {}


def kernel(x, W_r, b_r, W_u, b_u, W_c, b_c):
    if "nc" not in _CACHED:
        _CACHED["nc"] = build_program()
    nc = _CACHED["nc"]
    in_maps = prep_inputs(x, W_r, b_r, W_u, b_u, W_c, b_c)
    res = bass_utils.run_bass_kernel_spmd(
        nc, in_maps, core_ids=list(range(NC)), trace=False)
    _CACHED["last_results"] = res
    out = np.empty((B, T, H), dtype=np.float32)
    for c in range(NC):
        yc = res.results[c]["y"]                    # [H, T, BL] fp16
        out[c * BL:(c + 1) * BL] = (
            yc.astype(np.float32).transpose(2, 1, 0))
    return out
